# revision 24
# baseline (speedup 1.0000x reference)
"""Trainium2 Bass kernel for nn_Binary_CNN2 (binarized CNN, eval mode).

Data-parallel over 8 NeuronCores: batch 4096 -> 512 per core.

v3 pipeline per core (2-phase software pipeline, ping-pong buffers):
  s0:   x [512,1,28,28] f32 -> sign {+-0.5} bf16 -> DMA-transpose ->
        padded DRAM image xpad[ph][i' 34, j' 32, b 512] bf16
  conv: per lam: 6 HWDGE DMAs build rhs_t [36=(dx,dy,g), 2=r, 28=j, b]
        bf16 from xpad (no casts, no SWDGE); per (lam, jp): 4 matmuls
        N=512 (block-diag wc bf16, K=36) -> psq [128=(g,o), 4=(r,s), b];
        epilogue = maxpool+threshold-sign, alternating between
        R2 (ACT sign-first + DVE bf16 max tree -> a {+-1}) and
        R3 (DVE is_ge-first {+-0.5} + DVE max tree; the x2 is folded
        into the matching w2 columns on the host) -> a_sb[ph]
  fc1:  z1.T[h,b] = sum W2b.T @ a[ph]  (fp8 DoubleRow, exact int accum),
        BN2 affine + clip -> zt bf16 [128=h, 16=ht, 512=b]
  fc2:  logits[b,10] via bf16 matmuls + b3, log_softmax -> out [512,10]

Phase ph+1's s0/conv (DVE/ACT/DMA-heavy) overlaps phase ph's fc1
(PE-heavy): steady state is PE-bound.
"""

import numpy as np
import ml_dtypes

import concourse.bass as bass
import concourse.mybir as mybir
import concourse.tile as tile
from concourse import bacc
from concourse.bass_utils import run_bass_kernel_spmd

EPS = 1e-5
NCORES = 8
B = 512          # batch per core
H = 2048
C = 10
F32 = mybir.dt.float32
BF16 = mybir.dt.bfloat16
FP8 = mybir.dt.float8e4

UNROLL = 2       # phases per loop body (software pipeline depth)

# conv row-groups over the 28 image rows: sizes 8,8,8,4 (pool-pair aligned)
NPART_FOR_ILP = [128, 128, 96, 96]  # FC1 contraction rows valid per ilp

R3_EVERY = 2     # every R3_EVERY-th conv cell uses the DVE-first recipe
PIPELINE = False  # weave next phase's conv into fc1's PE stream

SIMPLIFY = set()


def _recipe(lam, jp):
    """Epilogue recipe for conv cell (lam, jp): 'R2' ACT-first, 'R3' DVE."""
    ci = lam * 14 + jp
    return "R3" if ci % R3_EVERY == (R3_EVERY - 1) else "R2"


def _f(c, k):
    """FC1 feature index map: chunk c=(ilp*14+jp), row k=(g*32+o) -> flat f."""
    ilp, jp = divmod(c, 14)
    g, o = divmod(k, 32)
    if g < 3:
        ip = 4 * g + ilp
    else:
        if ilp >= 2:
            return None
        ip = 12 + ilp
    return o * 196 + ip * 14 + jp


def build_nc(loop_n=None, parts=("s0", "conv", "fc1", "fc2"), simplify=None):
    simplify = SIMPLIFY if simplify is None else set(simplify)
    nc = bacc.Bacc("TRN2", target_bir_lowering=False, debug=False,
                   num_devices=NCORES)

    xin = nc.dram_tensor("x", [B, 28 * 28], F32, kind="ExternalInput")
    wc16 = nc.dram_tensor("wc16", [36, 128], BF16, kind="ExternalInput")
    negt1 = nc.dram_tensor("negt1", [128, 1], F32, kind="ExternalInput")
    post1 = nc.dram_tensor("post1", [128, 1], F32, kind="ExternalInput")
    w2b = nc.dram_tensor("w2b", [16, 128, 56, 128], FP8, kind="ExternalInput")
    s2t = nc.dram_tensor("s2t", [128, 16], F32, kind="ExternalInput")
    t2t = nc.dram_tensor("t2t", [128, 16], F32, kind="ExternalInput")
    w3t = nc.dram_tensor("w3t", [16, 128, C], BF16, kind="ExternalInput")
    b3r = nc.dram_tensor("b3r", [128, C], F32, kind="ExternalInput")
    out = nc.dram_tensor("out", [B, C], F32, kind="ExternalOutput")

    # padded transposed images, one per phase
    xpads = [nc.dram_tensor(f"xpad{p}", [34 * 32 * B], BF16, kind="Internal")
             for p in range(UNROLL)]

    hw_q = [nc.sync, nc.scalar]  # two HWDGE issue queues

    with tile.TileContext(nc) as tc:
        with (
            tc.tile_pool(name="consts", bufs=1) as consts,
            tc.tile_pool(name="persist", bufs=1) as persist,
            tc.tile_pool(name="s0p", bufs=1) as s0p,
            tc.tile_pool(name="rhsp", bufs=1) as rhsp,
            tc.tile_pool(name="epi", bufs=2) as epi,
            tc.tile_pool(name="w2pool", bufs=2) as w2p,
            tc.tile_pool(name="cctmp", bufs=2) as cct,
            tc.tile_pool(name="cpsum", bufs=1, space="PSUM") as cpsum,
            tc.tile_pool(name="zpsum", bufs=2, space="PSUM") as zps,
            tc.tile_pool(name="lpsum", bufs=1, space="PSUM") as lps,
        ):
            # ---- constants to SBUF (outside any timing loop) ----
            wc_sb = consts.tile([36, 128], BF16)
            nc.sync.dma_start(wc_sb[:], wc16.ap())
            negt1_sb = consts.tile([128, 1], F32)
            nc.sync.dma_start(negt1_sb[:], negt1.ap())
            post1_sb = consts.tile([128, 1], F32)
            nc.sync.dma_start(post1_sb[:], post1.ap())
            s2_sb = consts.tile([128, 16], F32)
            nc.sync.dma_start(s2_sb[:], s2t.ap())
            t2_sb = consts.tile([128, 16], F32)
            nc.sync.dma_start(t2_sb[:], t2t.ap())
            w3_sb = consts.tile([128, 16, C], BF16)
            nc.sync.dma_start(w3_sb[:], w3t.ap().rearrange("t p c -> p t c"))
            b3_sb = consts.tile([128, C], F32)
            nc.sync.dma_start(b3_sb[:], b3r.ap())

            a_sbs = [persist.tile([128, 4, 14, B], FP8, name=f"a_sb{p}")
                     for p in range(UNROLL)]
            zt_sb = persist.tile([128, 16, B], BF16)
            out_sb = persist.tile([128, 4, C], F32)

            def _body_s0(ph):
                xpad = xpads[ph]
                x_sb = s0p.tile([128, 4, 28 * 28], F32, tag="x")
                nc.sync.dma_start(
                    x_sb[:], xin.ap().rearrange("(bo p) f -> p bo f", p=128))
                xb_sb = s0p.tile([128, 4, 28, 32], BF16, tag="xb")
                nc.vector.memset(xb_sb[:], 0.0)
                # sign: (x >= 0) - 0.5 -> {+0.5, -0.5}; conv weights carry x2
                nc.vector.tensor_scalar(
                    xb_sb[:, :, :, 0:28],
                    x_sb[:].rearrange("p bo (h w) -> p bo h w", h=28),
                    0.0, 0.5, mybir.AluOpType.is_ge, mybir.AluOpType.subtract)

                # zero the whole xpad buffer (borders stay 0)
                zeros_sb = s0p.tile([128, 544], BF16, tag="zeros")
                nc.vector.memset(zeros_sb[:], 0.0)
                for q in range(8):
                    hw_q[q % 2].dma_start(
                        bass.AP(xpad, q * 128 * 544,
                                [[544, 128], [1, 544]]),
                        zeros_sb[:])

                # transpose b <-> (i,j32) in 128x128 tiles, on both HWDGE qs
                xT_sb = s0p.tile([128, 7, 4, 128], BF16, tag="xT")
                for c in range(7):
                    for bo in range(4):
                        src = xb_sb[:, bo].rearrange("p h w -> p (h w)")
                        hw_q[(c * 4 + bo) % 2].dma_start(
                            xT_sb[:, c, bo, :],
                            src[:, c * 128:(c + 1) * 128],
                            transpose=True)
                # write interior of xpad at element offset 33*512
                nc.sync.dma_start(
                    bass.AP(xpad, 33 * B,
                            [[B, 128], [128 * B, 7], [128, 4], [1, 128]]),
                    xT_sb[:])

            def _conv_epilogue(psq, a_slice, lam, jp):
                if _recipe(lam, jp) == "R2":
                    # ACT sign over all 4 slots, DVE bf16 max tree -> +-1
                    sq = epi.tile([128, 4, B], BF16, tag="sq")
                    nc.scalar.activation(
                        sq[:], psq[:],
                        mybir.ActivationFunctionType.Sign,
                        bias=negt1_sb[:])
                    g1 = epi.tile([128, 2, B], BF16, tag="g1")
                    nc.vector.tensor_tensor(
                        g1[:], sq[:, 0:2, :], sq[:, 2:4, :],
                        mybir.AluOpType.max)
                    nc.vector.tensor_tensor(
                        a_slice, g1[:, 0, :], g1[:, 1, :],
                        mybir.AluOpType.max)
                else:
                    # DVE is_ge-sub over 4 slots (-> +-0.5), DVE max tree;
                    # the x2 is folded into w2's columns for these cells
                    sq = epi.tile([128, 4, B], BF16, tag="sq3")
                    nc.vector.tensor_scalar(
                        sq[:], psq[:], post1_sb[:], 0.5,
                        mybir.AluOpType.is_ge, mybir.AluOpType.subtract)
                    g1 = epi.tile([128, 2, B], BF16, tag="g13")
                    nc.vector.tensor_tensor(
                        g1[:], sq[:, 0:2, :], sq[:, 2:4, :],
                        mybir.AluOpType.max)
                    nc.vector.tensor_tensor(
                        a_slice, g1[:, 0, :], g1[:, 1, :],
                        mybir.AluOpType.max)

            def _conv_step(ph, step):
                """Emit one conv step: ('build', lam) or ('cell', lam, jp)."""
                xpad = xpads[ph]
                a_sb = a_sbs[ph]
                if step[0] == "build":
                    lam = step[1]
                    rhs_t = rhsp.tile([36, 2, 28, B], BF16, tag="rhs")
                    _rhs_tiles[ph] = rhs_t
                    # one HWDGE DMA per (dx, r): partitions (dy,g), free
                    # (j, b) = one contiguous 28*B run per row
                    for dx in range(3):
                        for r in range(2):
                            off = (2 * lam + r) * 32 * B + dx * B
                            srcap = bass.AP(
                                xpad, off,
                                [[32 * B, 3], [8 * 32 * B, 4],
                                 [1, 28 * B]])
                            hw_q[(2 * dx + r) % 2].dma_start(
                                rhs_t[12 * dx:12 * dx + 12, r], srcap)
                else:
                    _, lam, jp = step
                    rhs_t = _rhs_tiles[ph]
                    psq = cpsum.tile([128, 4, B], F32, tag="cq")
                    for r in range(2):
                        for s in range(2):
                            nc.tensor.matmul(
                                psq[:, 2 * r + s, :],
                                wc_sb[:],
                                rhs_t[:, r, 2 * jp + s, :],
                                start=True, stop=True)
                    if "noepi" not in simplify:
                        _conv_epilogue(psq, a_sb[:, lam, jp, :], lam, jp)

            _rhs_tiles = {}

            def _conv_steps():
                steps = []
                for lam in range(4):
                    steps.append(("build", lam))
                    for jp in range(14):
                        steps.append(("cell", lam, jp))
                return steps

            def _body_conv(ph):
                for step in _conv_steps():
                    _conv_step(ph, step)

            def _body_fc1(ph, weave_ph=None):
                """FC1 on phase ph; optionally weave conv steps of weave_ph
                into the PE stream so they execute during fc1."""
                a_sb = a_sbs[ph]
                wsteps = _conv_steps() if weave_ph is not None else []
                wi = 0
                for ht in range(16):
                    w2_sb = w2p.tile([128, 56, 128], FP8, tag="w2")
                    # split the 0.92MB load across both HWDGE queues
                    nc.sync.dma_start(w2_sb[:, 0:28, :], w2b.ap()[ht, :, 0:28])
                    nc.scalar.dma_start(w2_sb[:, 28:56, :],
                                        w2b.ap()[ht, :, 28:56])
                    psz = zps.tile([128, B], F32, tag="z")
                    for cp in range(28):
                        lam, jph = divmod(cp, 7)
                        jp = 2 * jph
                        c = lam * 14 + jp
                        kk = NPART_FOR_ILP[lam]
                        nc.tensor.matmul(
                            psz[:],
                            w2_sb[0:kk, c:c + 2, :],
                            a_sb[0:kk, lam, jp:jp + 2, :],
                            start=(cp == 0), stop=(cp == 27),
                            perf_mode=mybir.MatmulPerfMode.DoubleRow)
                        # weave one conv step of the next phase after
                        # every 7th fc1 matmul (smooth PE pacing)
                        if cp % 7 == 6 and wi < len(wsteps):
                            _conv_step(weave_ph, wsteps[wi])
                            wi += 1
                    nc.scalar.activation(
                        zt_sb[:, ht, :], psz[:],
                        mybir.ActivationFunctionType.Identity,
                        bias=t2_sb[:, ht:ht + 1],
                        scale=s2_sb[:, ht:ht + 1])
                    nc.vector.tensor_scalar(
                        zt_sb[:, ht, :], zt_sb[:, ht, :],
                        1.0, -1.0, mybir.AluOpType.min, mybir.AluOpType.max)
                while wi < len(wsteps):
                    _conv_step(weave_ph, wsteps[wi])
                    wi += 1

            def _body_fc2(ph):
                for bt in range(4):
                    psl = lps.tile([128, C], F32, tag="l")
                    for ht in range(16):
                        nc.tensor.matmul(
                            psl[:],
                            zt_sb[:, ht, bt * 128:(bt + 1) * 128],
                            w3_sb[:, ht, :],
                            start=(ht == 0), stop=(ht == 15))
                    lg = cct.tile([128, C], F32, tag="lg")
                    nc.vector.tensor_add(lg[:], psl[:], b3_sb[:])
                    m = cct.tile([128, 1], F32, tag="m")
                    nc.vector.reduce_max(m[:], lg[:],
                                         axis=mybir.AxisListType.X)
                    negm = cct.tile([128, 1], F32, tag="negm")
                    nc.vector.tensor_scalar_mul(negm[:], m[:], -1.0)
                    e = cct.tile([128, C], F32, tag="e")
                    nc.scalar.activation(
                        e[:], lg[:], mybir.ActivationFunctionType.Exp,
                        bias=negm[:])
                    se = cct.tile([128, 1], F32, tag="se")
                    nc.vector.reduce_sum(se[:], e[:],
                                         axis=mybir.AxisListType.X)
                    lns = cct.tile([128, 1], F32, tag="lns")
                    nc.scalar.activation(
                        lns[:], se[:], mybir.ActivationFunctionType.Ln)
                    tot = cct.tile([128, 1], F32, tag="tot")
                    nc.vector.tensor_add(tot[:], m[:], lns[:])
                    nc.vector.tensor_scalar(
                        out_sb[:, bt, :], lg[:], tot[:], None,
                        mybir.AluOpType.subtract)

            def _tail(ph):
                if "fc2" in parts:
                    _body_fc2(ph)
                else:
                    nc.vector.memset(out_sb[:], 0.0)
                nc.sync.dma_start(
                    out.ap().rearrange("(bo p) c -> p bo c", p=128),
                    out_sb[:])

            def body_simple(ph):
                if "s0" in parts:
                    _body_s0(ph)
                if "conv" in parts:
                    _body_conv(ph)
                    if "noepi" in simplify:
                        nc.vector.memset(a_sbs[ph][:], 1.0)
                elif "fc1" in parts:
                    nc.vector.memset(a_sbs[ph][:], 1.0)  # ablation filler
                if "fc1" in parts:
                    _body_fc1(ph)
                elif "fc2" in parts:
                    nc.vector.memset(zt_sb[:], 0.5)  # ablation filler
                _tail(ph)

            full = all(p in parts for p in ("s0", "conv", "fc1", "fc2"))
            if loop_n is None:
                body_simple(0)
            elif loop_n < 0:
                # sim-only: manually unrolled pipeline, no hardware loop
                _body_s0(0)
                _body_conv(0)
                for it in range(-loop_n):
                    for ph in range(UNROLL):
                        nxt = (ph + 1) % UNROLL
                        _body_s0(nxt)
                        _body_fc1(ph, weave_ph=nxt)
                        _tail(ph)
            elif not full or not PIPELINE or "noepi" in simplify:
                with tc.For_i(0, loop_n, 1):
                    for ph in range(UNROLL):
                        body_simple(ph)
            else:
                # software pipeline: conv of phase ph+1 is woven into the
                # PE stream of fc1(ph) so DVE/ACT epilogue overlaps PE
                _body_s0(0)
                _body_conv(0)
                with tc.For_i(0, loop_n, 1):
                    for ph in range(UNROLL):
                        nxt = (ph + 1) % UNROLL
                        _body_s0(nxt)
                        _body_fc1(ph, weave_ph=nxt)
                        _tail(ph)

    nc.finalize()
    return nc


_NC_CACHE = {}


def _get_nc(loop_n=None, parts=("s0", "conv", "fc1", "fc2")):
    key = (loop_n, tuple(parts), tuple(sorted(SIMPLIFY)), R3_EVERY, PIPELINE)
    if key not in _NC_CACHE:
        _NC_CACHE[key] = build_nc(loop_n, parts)
    return _NC_CACHE[key]


def _host_prep(W1, b1, g1, be1, m1, v1, W2, b2, g2, be2, m2, v2, W3, b3):
    """Precompute small device-side constant tensors (numpy)."""
    s1 = (g1 / np.sqrt(v1 + EPS)).astype(np.float32)
    assert np.all(s1 != 0)
    # bn1 >= 0  <=>  sign(conv_nb - t1[o]) == sign(s1[o]); fold sign(s1)
    # into W2's columns so the device only computes sign(conv_nb - t1)
    t1 = (m1 - be1 / s1 - b1).astype(np.float32)
    sgn1 = np.where(s1 >= 0, 1.0, -1.0).astype(np.float32)
    negt1 = np.repeat(-t1[None, :], 4, axis=0).reshape(128, 1)
    post1 = (-negt1).copy()

    wc = np.zeros((36, 128), np.float32)
    w1s = np.where(W1[:, 0] >= 0, 2.0, -2.0).astype(np.float32)  # [32,3,3] x2
    for dy in range(3):
        for dx in range(3):
            for g in range(4):
                p = dx * 12 + dy * 4 + g
                wc[p, g * 32:(g + 1) * 32] = w1s[:, dy, dx]
    wc16 = wc.astype(ml_dtypes.bfloat16)

    w2s = np.where(W2 >= 0, 1.0, -1.0).astype(np.float32)  # [H, F1]
    w2s = w2s * sgn1[np.arange(w2s.shape[1]) // 196][None, :]
    w2bp = np.zeros((16, 128, 56, 128), np.float32)  # [ht, k, c, hh]
    for c in range(56):
        ilp, jp = divmod(c, 14)
        # R3 epilogue cells produce +-0.5; fold the missing x2 here
        cscale = 2.0 if _recipe(ilp, jp) == "R3" else 1.0
        for g in range(4):
            if _f(c, g * 32) is None:
                continue
            ip = 4 * g + ilp if g < 3 else 12 + ilp
            fs = np.arange(32) * 196 + ip * 14 + jp  # f for o=0..31
            blk = w2s[:, fs].reshape(16, 128, 32)   # [ht, hh, o]
            w2bp[:, g * 32:(g + 1) * 32, c, :] = \
                cscale * blk.transpose(0, 2, 1)
    w2bp = w2bp.astype(ml_dtypes.float8_e4m3)

    s2 = (g2 / np.sqrt(v2 + EPS)).astype(np.float32)
    t2 = (be2 + s2 * (b2 - m2)).astype(np.float32)
    s2t = s2.reshape(16, 128).T.copy()
    t2t = t2.reshape(16, 128).T.copy()

    w3t = np.ascontiguousarray(W3.T.astype(np.float32)).reshape(16, 128, C)
    w3t = w3t.astype(ml_dtypes.bfloat16)
    b3r = np.repeat(b3[None, :].astype(np.float32), 128, axis=0)
    return dict(wc16=wc16, negt1=negt1, post1=post1, w2b=w2bp, s2t=s2t,
                t2t=t2t, w3t=w3t, b3r=np.ascontiguousarray(b3r))


def _make_in_maps(x, consts):
    xs = np.asarray(x, np.float32).reshape(NCORES, B, 28 * 28)
    in_maps = []
    for i in range(NCORES):
        m = {"x": np.ascontiguousarray(xs[i])}
        m.update(consts)
        in_maps.append(m)
    return in_maps


def _prep_all(inputs):
    names = ["W1", "b1", "g1", "be1", "m1", "v1", "W2", "b2", "g2", "be2",
             "m2", "v2", "W3", "b3"]
    return _host_prep(*[np.asarray(inputs[n], np.float32) for n in names])


def kernel(x, **weights):
    consts = _prep_all(weights)
    nc = _get_nc(None)
    in_maps = _make_in_maps(x, consts)
    res = run_bass_kernel_spmd(nc, in_maps, core_ids=list(range(NCORES)))
    outs = [res.results[i]["out"] for i in range(NCORES)]
    return np.concatenate(outs, axis=0).astype(np.float32)


def _make_runner(nc, in_maps):
    """Build a reusable executor with inputs resident on device (no re-upload)."""
    import jax
    import jax.numpy as jnp
    from jax.sharding import Mesh, PartitionSpec, NamedSharding
    from jax.experimental.shard_map import shard_map
    from concourse import bass2jax
    from concourse.bass2jax import _bass_exec_p, install_neuronx_cc_hook

    install_neuronx_cc_hook()
    n_cores = len(in_maps)
    partition_name = nc.partition_id_tensor.name if nc.partition_id_tensor else None
    in_names, out_names, out_avals, zero_outs = [], [], [], []
    for alloc in nc.m.functions[0].allocations:
        if not isinstance(alloc, mybir.MemoryLocationSet):
            continue
        name = alloc.memorylocations[0].name
        if alloc.kind == "ExternalInput":
            if name != partition_name:
                in_names.append(name)
        elif alloc.kind == "ExternalOutput":
            shape = tuple(alloc.tensor_shape)
            dtype = mybir.dt.np(alloc.dtype)
            out_names.append(name)
            out_avals.append(jax.core.ShapedArray(shape, dtype))
            zero_outs.append(np.zeros(shape, dtype))
    n_params = len(in_names)
    n_outs = len(out_avals)
    in_names.extend(out_names)
    if partition_name is not None:
        in_names.append(partition_name)
    donate = tuple(range(n_params, n_params + n_outs))

    def _body(*args):
        operands = list(args)
        if partition_name is not None:
            operands.append(bass2jax.partition_id_tensor())
        outs = _bass_exec_p.bind(
            *operands, out_avals=tuple(out_avals), in_names=tuple(in_names),
            out_names=tuple(out_names), lowering_input_output_aliases=(),
            sim_require_finite=True, sim_require_nnan=True, nc=nc)
        return tuple(outs)

    devices = jax.devices()[:n_cores]
    mesh = Mesh(np.asarray(devices), ("core",))
    sharded = jax.jit(
        shard_map(_body, mesh=mesh,
                  in_specs=(PartitionSpec("core"),) * (n_params + n_outs),
                  out_specs=(PartitionSpec("core"),) * n_outs,
                  check_rep=False),
        donate_argnums=donate, keep_unused=True)
    shard = NamedSharding(mesh, PartitionSpec("core"))
    per_core = [[np.asarray(m[nm]) for nm in in_names[:n_params]]
                for m in in_maps]
    dev_in = [jax.device_put(
                np.concatenate([per_core[c][i] for c in range(n_cores)],
                               axis=0), shard)
              for i in range(n_params)]
    concat_zero_shapes = [((n_cores * z.shape[0],) + z.shape[1:], z.dtype)
                          for z in zero_outs]

    def run():
        zeros = [jnp.zeros(s, d, device=shard) for s, d in concat_zero_shapes]
        outs = sharded(*dev_in, *zeros)
        jax.block_until_ready(outs)
        return outs

    return run


def measure_exec_ns(inputs, n_lo=2, n_hi=66, reps=11):
    """HW exec time per pipeline iteration via looped-kernel wall-clock delta.

    loop_n counts UNROLL-iteration bodies, so per-iteration time divides
    by UNROLL.
    """
    import time
    consts = _prep_all(inputs)
    in_maps = _make_in_maps(inputs["x"], consts)

    def med_time(loop_n):
        nc = _get_nc(loop_n, measure_exec_ns.parts)
        run = _make_runner(nc, in_maps)
        run()  # compile + warm
        ts = []
        for _ in range(reps):
            t0 = time.time()
            run()
            ts.append(time.time() - t0)
        ts.sort()
        return ts[len(ts) // 2], ts

    t_lo, all_lo = med_time(n_lo)
    t_hi, all_hi = med_time(n_hi)
    measure_exec_ns.last = (all_lo, all_hi)
    return (t_hi - t_lo) / ((n_hi - n_lo) * UNROLL) * 1e9


measure_exec_ns.parts = ("s0", "conv", "fc1", "fc2")
build_nc_looped = build_nc  # marker for test.py
